# revision 47
# baseline (speedup 1.0000x reference)
"""Trainium2 Bass kernel for nn_CausalAggregator.

Computes, for target stocks y:
    out[y, :] = Beta[:, y] @ concat([X, adjacency[:, y, :]], 1) @ W + bias
              = (Beta.T @ X) @ Wf  +  (einsum('ny,nyc->yc', Beta, adj)) @ Wa + bias

Sharding: split Beta / adjacency along the target axis y across 8 cores;
replicate X, weight, bias. Each core computes 512 output rows; no
cross-device reduction.

Per-core algorithm (N=4096 source stocks, Y=512 targets, D=O=256, C=3).
The kernel is HBM-bound (DMA queues share one ~360 GB/s pool), so every
wire tensor is downcast host-side (free) to fp8-e4m3.  X is loop-invariant
and replicated, so it is hoisted into SBUF once as a const; the per-rep
stream is row n = [beta(512B) | adj channel-major(1536B)], grouped GS n-tiles
per DMA with a tile-major permutation (contraction is order-invariant in n).
~8.4 MB/core vs 38 MB fp32.

All matmuls run in fp8 DoubleRow perf mode (K=256/pass, 0.5 cyc/row).
The einsum term runs on the PE as t-major diagonal-block matmuls: for each
y-block t, M_t += Beta_t^T @ [A0|A1|A2]_t accumulates a [128, 384] PSUM
tile whose three 128-col diagonals are the per-target channel sums; the
epilogue extracts them with one tiled-identity mask mul (DVE) per block,
then lands the adjacency term directly into the output accumulation as
rank-1 diag-mask @ broadcast-Wa matmuls (bias likewise via identity @
broadcast-bias), so there is no selector/stacking stage at all.

y-blocks 0,2 accumulate in one PSUM bank and 1,3 in another so a block's
drain (Activation/DVE alternating) never serializes the next block; each
block gets its own output DMA so the last quarter only waits on the last
copy.  PSUM start=True zeroes the whole 2KB bank, so only the FIRST
matmul touching each bank per rep carries start=True (M_START_ZERO)
instead of DVE memsets.

Queue discipline: the big input-stream DMAs ride the sync/scalar (HWDGE)
queues, consts ride the Pool queue, and output DMAs reuse sync/scalar
(idle by the tail).  The DMA engines serialize at ~360 GB/s, so bytes ~=
time: deep input buffering (IO_BUFS) keeps them saturated across rep
boundaries.  PE keep-warm filler matmuls (N_FILL) into the dead f banks
hold the PE at max p-state through DMA-paced idle gaps, which keeps the
post-stream matmul backlog at full clock.  Constants and PSUM
accumulators are hoisted out of the per-rep body so back-to-back
invocations pipeline.
"""

import numpy as np
import ml_dtypes

import concourse.mybir as mybir
import concourse.tile as tile
from concourse import bacc
from concourse.bass import ds, ts
from concourse.bass_utils import run_bass_kernel_spmd

P = 128
F32 = mybir.dt.float32
BF16 = mybir.dt.bfloat16
F8 = mybir.dt.float8e4
F16 = mybir.dt.float16
U8 = mybir.dt.uint8
DR = mybir.MatmulPerfMode.DoubleRow
COPY = mybir.ActivationFunctionType.Copy

# Full problem shapes (hardcoded; kernel.py must be self-contained).
N_FULL = 4096   # source stocks (contraction axis)
Y_TOTAL = 4096  # target stocks (sharded)
D_FULL = 256    # input features
O_FULL = 256    # output features
C_FULL = 3      # adjacency channels (weight rows = input_dim + 3)
N_CORES = 8
Y_FULL = Y_TOTAL // N_CORES  # per-core target slice
GS = 4          # n-tiles per DMA group (GSCHED overrides when set; mixed
# sizes let the tile scheduler reorder DMAs and wreck arrival order, so
# keep it uniform)
GSCHED = None
IO_BUFS = 10    # input stream multi-buffering
XT_CHUNKS = 4   # const X loads split so the first group isn't gated on 1MB

M_START_ZERO = True  # zero m_psum via first-matmul start=True bank clear
N_FILL = 60     # PE keep-warm fillers per group (p-state hold)
OUT_Q = "hw"    # output DMA queues: "hw" = sync/scalar HWDGE, "pool" = gpsimd
DBG_SKIP = frozenset()  # sim-only ablation: {"gt", "m", "epi", "f"}


def _layout(Y, D, C):
    """Byte offsets of the packed subtile row [beta | adj]."""
    off, lay = 0, {}
    for name, sz in (("beta", Y), ("adj", C * Y)):
        lay[name] = off
        off += sz
    return lay, off


def emit_body(tc, io, cst, N, Y, D, O, C):
    nc = tc.nc
    pkd, out = io["pkd"], io["out"]
    n_nt, n_yt, n_dt = N // P, Y // P, D // P
    lay, sub = _layout(Y, D, C)
    sched = GSCHED or [GS] * (n_nt // GS)
    assert sum(sched) == n_nt and all(g % 2 == 0 for g in sched)
    n_grp = len(sched)
    n_pairs = n_nt // 2

    iopool, fpool, opool = cst["iopool"], cst["fpool"], cst["opool"]
    gt_psum, m_psum, f_psum = cst["gt_psum"], cst["m_psum"], cst["f_psum"]
    wf_t, wa_bc, bias_bc, id_t, xt = (
        cst["wf_t"], cst["wa_bc"], cst["bias_bc"], cst["id_t"], cst["xt"])

    gt_sb = [fpool.tile([P, Y], BF16, tag=f"gt{d_t}", name=f"gt{d_t}")
             for d_t in range(n_dt)]
    mask_sb = [fpool.tile([P, C * P], BF16, tag=f"mask{t}", name=f"mask{t}")
               for t in range(n_yt)]

    if not M_START_ZERO:
        for t in range(n_yt):
            nc.vector.memset(m_psum[t], 0.0)

    queues = [nc.sync, nc.scalar]
    s0 = 0
    for g, gs in enumerate(sched):
        pk_t = iopool.tile([P, gs * sub], U8, tag=f"pk{gs}", name=f"pk{gs}")
        queues[g % 2].dma_start(out=pk_t, in_=pkd[:, ds(s0 * sub, gs * sub)])

        for jj in range(gs // 2):
            pi = s0 // 2 + jj
            first = pi == 0
            last = pi == n_pairs - 1
            pair = pk_t[:, ds(jj * 2 * sub, 2 * sub)].rearrange(
                "p (i b) -> p i b", i=2)
            beta_p = pair[:, :, ds(lay["beta"], Y)].bitcast(F8)
            adj_p = pair[:, :, ds(lay["adj"], C * Y)].bitcast(F8)
            x_p = xt[:, ds(pi * 2 * D, 2 * D)].rearrange(
                "p (i b) -> p i b", i=2).bitcast(F8)

            if "gt" not in DBG_SKIP:
                for d_t in range(n_dt):
                    nc.tensor.matmul(gt_psum[d_t], x_p[:, :, ts(d_t, P)],
                                     beta_p, start=first, stop=last,
                                     perf_mode=DR)
            if "m" not in DBG_SKIP:
                # t-major: one matmul per y-block covers all 3 channels
                # (adj is packed [t][c][128] within each row)
                for t in range(n_yt):
                    nc.tensor.matmul(
                        m_psum[t][:, 0:C * P],
                        beta_p[:, :, ts(t, P)],
                        adj_p[:, :, ds(t * C * P, C * P)],
                        start=(M_START_ZERO and first),
                        stop=last, perf_mode=DR,
                        skip_group_check=True)
            # keep-warm fillers: PE idles waiting on the next group's DMA,
            # which drops it out of max p-state; cheap no-dep matmuls into
            # the (dead until epilogue) f banks keep the clock up.  None
            # after the last group so the real tail is never delayed.
            if "f" not in DBG_SKIP and jj == gs // 2 - 1 and g < n_grp - 2:
                for k in range(N_FILL):
                    nc.tensor.matmul(
                        f_psum[g % 2][:, ds((k % 4) * P, P)],
                        x_p[:, :, 0:P], x_p[:, :, 0:P], start=True,
                        stop=False, perf_mode=DR, skip_group_check=True)
        s0 += gs

    if "epi" in DBG_SKIP:
        return
    # drain accumulators to SBUF: gt on Activation, per-y-block masks on
    # DVE so each output block unblocks as early as possible
    H = Y // 2
    for h in range(2):
        for d_t in range(n_dt):
            nc.scalar.activation(gt_sb[d_t][:, ds(h * H, H)],
                                 gt_psum[d_t][:, ds(h * H, H)], COPY)
    for t in range(n_yt):
        nc.vector.tensor_mul(mask_sb[t], m_psum[t][:, 0:C * P],
                             id_t[:, 0:C * P])

    # y-blocks 0,2 accumulate in psum bank A, 1,3 in bank B so a block's
    # drain never serializes against the next block's matmuls; the first
    # matmul touching each bank carries start=True (whole-bank zero).  The
    # adjacency term lands as rank-1 diag-mask @ broadcast-Wa matmuls; bias
    # likewise via identity @ broadcast-bias.
    o_sb = opool.tile([P, n_yt * O], F32, tag="osb", name="osb")
    # per y-half: bias+feat matmuls (gated on that half's gt drain), then
    # the mask-gated adjacency rank-1 terms, copy, and per-block output
    # DMA — so PE's in-order queue never parks later-gated work in front
    # of an unblocked block
    for half in range(2):
        for y_t in (2 * half, 2 * half + 1):
            fp = f_psum[y_t % 2][:, ds((y_t // 2) * O, O)]
            nc.tensor.matmul(fp, id_t[:, 0:P], bias_bc,
                             start=(y_t < 2), stop=False,
                             skip_group_check=True)
            for d_t in range(n_dt):
                nc.tensor.matmul(fp, gt_sb[d_t][:, ts(y_t, P)],
                                 wf_t[d_t], start=False,
                                 stop=False, skip_group_check=True)
        for y_t in (2 * half, 2 * half + 1):
            fp = f_psum[y_t % 2][:, ds((y_t // 2) * O, O)]
            for c in range(C):
                nc.tensor.matmul(fp, mask_sb[y_t][:, ts(c, P)], wa_bc[c],
                                 start=False, stop=(c == C - 1),
                                 skip_group_check=True)
            if y_t % 2:
                nc.vector.tensor_copy(o_sb[:, ds(y_t * O, O)], fp)
            else:
                nc.scalar.activation(o_sb[:, ds(y_t * O, O)], fp, COPY)
            oq = ((nc.sync, nc.scalar) if OUT_Q == "hw"
                  else (nc.gpsimd, nc.gpsimd))[y_t % 2]
            oq.dma_start(
                out=out[ts(y_t, P), :], in_=o_sb[:, ds(y_t * O, O)])


def emit_kernel(tc, io, N, Y, D, O, C, reps=1):
    nc = tc.nc
    n_dt = D // P
    n_nt = N // P
    n_yt = Y // P
    with (
        tc.tile_pool(name="const", bufs=1) as cpool,
        tc.tile_pool(name="io", bufs=IO_BUFS) as iopool,
        tc.tile_pool(name="fin", bufs=2) as fpool,
        tc.tile_pool(name="osb", bufs=2) as opool,
        tc.tile_pool(name="acc", bufs=1, space="PSUM") as accpool,
    ):
        cst = {"iopool": iopool, "fpool": fpool, "opool": opool}
        # X^T stream hoisted into SBUF once: [128, n_nt * D] fp8 bytes,
        # subtile s holds rows [s*128, (s+1)*128) of X
        xt = cpool.tile([P, n_nt * D], U8, tag="xt", name="xt")
        ck = n_nt * D // XT_CHUNKS
        for k in range(XT_CHUNKS):
            nc.gpsimd.dma_start(out=xt[:, ds(k * ck, ck)],
                                in_=io["xt"][:, ds(k * ck, ck)])
        # [I I I I] tiled identity
        id_t = cpool.tile([P, C * P], BF16, tag="ident", name="ident")
        nc.gpsimd.dma_start(out=id_t, in_=io["ident"])
        wf_t = []
        for d_t in range(n_dt):
            t = cpool.tile([P, O], BF16, tag=f"wf{d_t}", name=f"wf{d_t}")
            nc.gpsimd.dma_start(out=t, in_=io["wf"][ts(d_t, P), :])
            wf_t.append(t)
        # Wa rows / bias broadcast across partitions for the rank-1 epilogue
        wa_bc = []
        for c in range(C):
            t = cpool.tile([P, O], BF16, tag=f"wab{c}", name=f"wab{c}")
            nc.gpsimd.dma_start(
                out=t, in_=io["wa"][c:c + 1, :].to_broadcast((P, O)))
            wa_bc.append(t)
        bias_bc = cpool.tile([P, O], BF16, tag="bias", name="bias")
        nc.gpsimd.dma_start(
            out=bias_bc, in_=io["bias"].unsqueeze(0).to_broadcast((P, O)))
        cst.update(wf_t=wf_t, id_t=id_t, xt=xt, wa_bc=wa_bc, bias_bc=bias_bc)

        # shared PSUM: 2 gt + 4 m + 2 f = 8 banks
        cst["gt_psum"] = [accpool.tile([P, Y], F32, tag=f"gtp{d}", name=f"gtp{d}")
                          for d in range(n_dt)]
        cst["m_psum"] = [accpool.tile([P, C * P], F32, tag=f"mp{t}", name=f"mp{t}")
                         for t in range(n_yt)]
        cst["f_psum"] = [accpool.tile([P, n_yt * O // 2], F32, tag=f"fp{b}",
                                      name=f"fp{b}") for b in range(2)]

        for _ in range(reps):
            emit_body(tc, io, cst, N, Y, D, O, C)


def build_nc(N=N_FULL, Y=Y_FULL, D=D_FULL, O=O_FULL, C=C_FULL, reps=1,
             internal_inputs=False):
    nc = bacc.Bacc("TRN2", target_bir_lowering=False, debug=False)
    kind = "Internal" if internal_inputs else "ExternalInput"
    _, sub = _layout(Y, D, C)
    io = {
        "pkd": nc.dram_tensor("pkd", [P, (N // P) * sub], U8, kind=kind).ap(),
        "xt": nc.dram_tensor("xt", [P, (N // P) * D], U8, kind=kind).ap(),
        "wf": nc.dram_tensor("wf", [D, O], BF16, kind=kind).ap(),
        "wa": nc.dram_tensor("wa", [C, O], BF16, kind=kind).ap(),
        "bias": nc.dram_tensor("bias", [O], BF16, kind=kind).ap(),
        "ident": nc.dram_tensor("ident", [P, C * P], BF16, kind=kind).ap(),
        "out": nc.dram_tensor("out", [Y, O], F32, kind="ExternalOutput").ap(),
    }
    with tile.TileContext(nc) as tc:
        emit_kernel(tc, io, N, Y, D, O, C, reps=reps)
    nc.compile()
    return nc


_NC_CACHE = None


def _get_nc():
    global _NC_CACHE
    if _NC_CACHE is None:
        _NC_CACHE = build_nc()
    return _NC_CACHE


E4M3 = ml_dtypes.float8_e4m3


def _q8(a):
    return np.ascontiguousarray(a).astype(E4M3)


def run(adjacency, input_feature, Beta, weight, bias, trace=False):
    nc = _get_nc()
    adjacency = np.asarray(adjacency, dtype=np.float32)
    input_feature = np.asarray(input_feature, dtype=np.float32)
    Beta = np.asarray(Beta, dtype=np.float32)
    weight = np.ascontiguousarray(np.asarray(weight, dtype=np.float32))
    bias = np.ascontiguousarray(np.asarray(bias, dtype=np.float32))

    x8 = _q8(input_feature)
    # xt[p, s*D + d] = X8[s*128 + p, d]
    xt = np.ascontiguousarray(
        x8.view(np.uint8).reshape(N_FULL // P, P, D_FULL)
          .transpose(1, 0, 2).reshape(P, -1))
    wf = np.ascontiguousarray(weight[:D_FULL]).astype(ml_dtypes.bfloat16)
    wa = np.ascontiguousarray(weight[D_FULL:]).astype(ml_dtypes.bfloat16)
    bias16 = bias.astype(ml_dtypes.bfloat16)
    ident = np.ascontiguousarray(
        np.tile(np.eye(P, dtype=ml_dtypes.bfloat16), (1, C_FULL)))

    in_maps = []
    for i in range(N_CORES):
        ys = slice(i * Y_FULL, (i + 1) * Y_FULL)
        beta8 = _q8(Beta[:, ys])
        # [n][t][c][128] so one matmul per y-block covers all 3 channels
        adj8 = _q8(adjacency[:, ys, :]
                   .reshape(N_FULL, Y_FULL // P, P, C_FULL)
                   .transpose(0, 1, 3, 2)).reshape(N_FULL, -1)
        row = np.concatenate(
            [beta8.view(np.uint8), adj8.view(np.uint8)], axis=1)  # [N, sub]
        sub = row.shape[1]
        # pkd[p, s*sub + b] = row[s*128 + p, b]: any tile-aligned group is
        # a contiguous column slice
        pkd = np.ascontiguousarray(
            row.reshape(N_FULL // P, P, sub)
               .transpose(1, 0, 2).reshape(P, -1))
        in_maps.append({
            "pkd": pkd,
            "xt": xt,
            "wf": wf,
            "wa": wa,
            "bias": bias16,
            "ident": ident,
        })
    res = run_bass_kernel_spmd(nc, in_maps, core_ids=list(range(N_CORES)),
                               trace=trace)
    out = np.concatenate([res.results[i]["out"] for i in range(N_CORES)],
                         axis=0).astype(np.float32)
    return out, res


def kernel(adjacency, input_feature, Beta, weight, bias):
    out, _ = run(adjacency, input_feature, Beta, weight, bias, trace=False)
    return out


# revision 48
# speedup vs baseline: 4.4953x; 4.4953x over previous
"""Trainium2 Bass kernel for nn_CausalAggregator.

Computes, for target stocks y:
    out[y, :] = Beta[:, y] @ concat([X, adjacency[:, y, :]], 1) @ W + bias
              = (Beta.T @ X) @ Wf  +  (einsum('ny,nyc->yc', Beta, adj)) @ Wa + bias

Sharding: split Beta / adjacency along the target axis y across 8 cores;
replicate X, weight, bias. Each core computes 512 output rows; no
cross-device reduction.

Per-core algorithm (N=4096 source stocks, Y=512 targets, D=O=256, C=3).
The kernel is HBM-bound (DMA queues share one ~360 GB/s pool), so every
wire tensor is downcast host-side (free) to fp8-e4m3.  X is loop-invariant
and replicated, so it is hoisted into SBUF once as a const; the per-rep
stream is row n = [beta(512B) | adj channel-major(1536B)], grouped GS n-tiles
per DMA with a tile-major permutation (contraction is order-invariant in n).
~8.4 MB/core vs 38 MB fp32.

All matmuls run in fp8 DoubleRow perf mode (K=256/pass, 0.5 cyc/row).
The einsum term runs on the PE as t-major diagonal-block matmuls: for each
y-block t, M_t += Beta_t^T @ [A0|A1|A2]_t accumulates a [128, 384] PSUM
tile whose three 128-col diagonals are the per-target channel sums; the
epilogue extracts them with one tiled-identity mask mul (DVE) per block,
then lands the adjacency term directly into the output accumulation as
rank-1 diag-mask @ broadcast-Wa matmuls (bias likewise via identity @
broadcast-bias), so there is no selector/stacking stage at all.

y-blocks 0,2 accumulate in one PSUM bank and 1,3 in another so a block's
drain (Activation/DVE alternating) never serializes the next block; each
block gets its own output DMA so the last quarter only waits on the last
copy.  PSUM start=True zeroes the whole 2KB bank, so only the FIRST
matmul touching each bank per rep carries start=True (M_START_ZERO)
instead of DVE memsets.

Queue discipline: the big input-stream DMAs ride the sync/scalar (HWDGE)
queues, consts ride the Pool queue, and output DMAs reuse sync/scalar
(idle by the tail).  The DMA engines serialize at ~360 GB/s, so bytes ~=
time: deep input buffering (IO_BUFS) keeps them saturated across rep
boundaries.  PE keep-warm filler matmuls (N_FILL) into the dead f banks
hold the PE at max p-state through DMA-paced idle gaps, which keeps the
post-stream matmul backlog at full clock.  Constants and PSUM
accumulators are hoisted out of the per-rep body so back-to-back
invocations pipeline.
"""

import numpy as np
import ml_dtypes

import concourse.mybir as mybir
import concourse.tile as tile
from concourse import bacc
from concourse.bass import ds, ts
from concourse.bass_utils import run_bass_kernel_spmd

P = 128
F32 = mybir.dt.float32
BF16 = mybir.dt.bfloat16
F8 = mybir.dt.float8e4
F16 = mybir.dt.float16
U8 = mybir.dt.uint8
DR = mybir.MatmulPerfMode.DoubleRow
COPY = mybir.ActivationFunctionType.Copy

# Full problem shapes (hardcoded; kernel.py must be self-contained).
N_FULL = 4096   # source stocks (contraction axis)
Y_TOTAL = 4096  # target stocks (sharded)
D_FULL = 256    # input features
O_FULL = 256    # output features
C_FULL = 3      # adjacency channels (weight rows = input_dim + 3)
N_CORES = 8
Y_FULL = Y_TOTAL // N_CORES  # per-core target slice
GS = 4          # n-tiles per DMA group (GSCHED overrides when set; mixed
# sizes let the tile scheduler reorder DMAs and wreck arrival order, so
# keep it uniform)
GSCHED = None
IO_BUFS = 10    # input stream multi-buffering
XT_CHUNKS = 4   # const X loads split so the first group isn't gated on 1MB

M_START_ZERO = True  # zero m_psum via first-matmul start=True bank clear
N_FILL = 60     # PE keep-warm fillers per group (p-state hold)
OUT_Q = "hw"    # output DMA queues: "hw" = sync/scalar HWDGE, "pool" = gpsimd
DBG_SKIP = frozenset()  # sim-only ablation: {"gt", "m", "epi", "f"}


def _layout(Y, D, C):
    """Byte offsets of the packed subtile row [beta | adj]."""
    off, lay = 0, {}
    for name, sz in (("beta", Y), ("adj", C * Y)):
        lay[name] = off
        off += sz
    return lay, off


def emit_body(tc, io, cst, N, Y, D, O, C):
    nc = tc.nc
    pkd, out = io["pkd"], io["out"]
    n_nt, n_yt, n_dt = N // P, Y // P, D // P
    lay, sub = _layout(Y, D, C)
    sched = GSCHED or [GS] * (n_nt // GS)
    assert sum(sched) == n_nt and all(g % 2 == 0 for g in sched)
    n_grp = len(sched)
    n_pairs = n_nt // 2

    iopool, fpool, opool = cst["iopool"], cst["fpool"], cst["opool"]
    gt_psum, m_psum, f_psum = cst["gt_psum"], cst["m_psum"], cst["f_psum"]
    wf_t, wa_bc, bias_bc, id_t, xt = (
        cst["wf_t"], cst["wa_bc"], cst["bias_bc"], cst["id_t"], cst["xt"])

    gt_sb = [fpool.tile([P, Y], BF16, tag=f"gt{d_t}", name=f"gt{d_t}")
             for d_t in range(n_dt)]
    mask_sb = [fpool.tile([P, C * P], BF16, tag=f"mask{t}", name=f"mask{t}")
               for t in range(n_yt)]

    if not M_START_ZERO:
        for t in range(n_yt):
            nc.vector.memset(m_psum[t], 0.0)

    queues = [nc.sync, nc.scalar]
    s0 = 0
    for g, gs in enumerate(sched):
        pk_t = iopool.tile([P, gs * sub], U8, tag=f"pk{gs}", name=f"pk{gs}")
        queues[g % 2].dma_start(out=pk_t, in_=pkd[:, ds(s0 * sub, gs * sub)])

        for jj in range(gs // 2):
            pi = s0 // 2 + jj
            first = pi == 0
            last = pi == n_pairs - 1
            pair = pk_t[:, ds(jj * 2 * sub, 2 * sub)].rearrange(
                "p (i b) -> p i b", i=2)
            beta_p = pair[:, :, ds(lay["beta"], Y)].bitcast(F8)
            adj_p = pair[:, :, ds(lay["adj"], C * Y)].bitcast(F8)
            x_p = xt[:, ds(pi * 2 * D, 2 * D)].rearrange(
                "p (i b) -> p i b", i=2).bitcast(F8)

            if "gt" not in DBG_SKIP:
                for d_t in range(n_dt):
                    nc.tensor.matmul(gt_psum[d_t], x_p[:, :, ts(d_t, P)],
                                     beta_p, start=first, stop=last,
                                     perf_mode=DR)
            if "m" not in DBG_SKIP:
                # t-major: one matmul per y-block covers all 3 channels
                # (adj is packed [t][c][128] within each row)
                for t in range(n_yt):
                    nc.tensor.matmul(
                        m_psum[t][:, 0:C * P],
                        beta_p[:, :, ts(t, P)],
                        adj_p[:, :, ds(t * C * P, C * P)],
                        start=(M_START_ZERO and first),
                        stop=last, perf_mode=DR,
                        skip_group_check=True)
            # keep-warm fillers: PE idles waiting on the next group's DMA,
            # which drops it out of max p-state; cheap no-dep matmuls into
            # the (dead until epilogue) f banks keep the clock up.  None
            # after the last group so the real tail is never delayed.
            if "f" not in DBG_SKIP and jj == gs // 2 - 1 and g < n_grp - 2:
                for k in range(N_FILL):
                    nc.tensor.matmul(
                        f_psum[g % 2][:, ds((k % 4) * P, P)],
                        x_p[:, :, 0:P], x_p[:, :, 0:P], start=True,
                        stop=False, perf_mode=DR, skip_group_check=True)
        s0 += gs

    if "epi" in DBG_SKIP:
        return
    # drain accumulators to SBUF: gt on Activation, per-y-block masks on
    # DVE so each output block unblocks as early as possible
    H = Y // 2
    for h in range(2):
        for d_t in range(n_dt):
            nc.scalar.activation(gt_sb[d_t][:, ds(h * H, H)],
                                 gt_psum[d_t][:, ds(h * H, H)], COPY)
    for t in range(n_yt):
        nc.vector.tensor_mul(mask_sb[t], m_psum[t][:, 0:C * P],
                             id_t[:, 0:C * P])

    # y-blocks 0,2 accumulate in psum bank A, 1,3 in bank B so a block's
    # drain never serializes against the next block's matmuls; the first
    # matmul touching each bank carries start=True (whole-bank zero).  The
    # adjacency term lands as rank-1 diag-mask @ broadcast-Wa matmuls; bias
    # likewise via identity @ broadcast-bias.
    o_sb = opool.tile([P, n_yt * O], F32, tag="osb", name="osb")
    # per y-half: bias+feat matmuls (gated on that half's gt drain), then
    # the mask-gated adjacency rank-1 terms, copy, and per-block output
    # DMA — so PE's in-order queue never parks later-gated work in front
    # of an unblocked block
    for half in range(2):
        for y_t in (2 * half, 2 * half + 1):
            fp = f_psum[y_t % 2][:, ds((y_t // 2) * O, O)]
            nc.tensor.matmul(fp, id_t[:, 0:P], bias_bc,
                             start=(y_t < 2), stop=False,
                             skip_group_check=True)
            for d_t in range(n_dt):
                nc.tensor.matmul(fp, gt_sb[d_t][:, ts(y_t, P)],
                                 wf_t[d_t], start=False,
                                 stop=False, skip_group_check=True)
        for y_t in (2 * half, 2 * half + 1):
            fp = f_psum[y_t % 2][:, ds((y_t // 2) * O, O)]
            for c in range(C):
                nc.tensor.matmul(fp, mask_sb[y_t][:, ts(c, P)], wa_bc[c],
                                 start=False, stop=(c == C - 1),
                                 skip_group_check=True)
            if y_t % 2:
                nc.vector.tensor_copy(o_sb[:, ds(y_t * O, O)], fp)
            else:
                nc.scalar.activation(o_sb[:, ds(y_t * O, O)], fp, COPY)
            oq = ((nc.sync, nc.scalar) if OUT_Q == "hw"
                  else (nc.gpsimd, nc.gpsimd))[y_t % 2]
            oq.dma_start(
                out=out[ts(y_t, P), :], in_=o_sb[:, ds(y_t * O, O)])


def emit_kernel(tc, io, N, Y, D, O, C, reps=1):
    nc = tc.nc
    n_dt = D // P
    n_nt = N // P
    n_yt = Y // P
    with (
        tc.tile_pool(name="const", bufs=1) as cpool,
        tc.tile_pool(name="io", bufs=IO_BUFS) as iopool,
        tc.tile_pool(name="fin", bufs=2) as fpool,
        tc.tile_pool(name="osb", bufs=2) as opool,
        tc.tile_pool(name="acc", bufs=1, space="PSUM") as accpool,
    ):
        cst = {"iopool": iopool, "fpool": fpool, "opool": opool}
        # X^T stream hoisted into SBUF once: [128, n_nt * D] fp8 bytes,
        # subtile s holds rows [s*128, (s+1)*128) of X
        xt = cpool.tile([P, n_nt * D], U8, tag="xt", name="xt")
        ck = n_nt * D // XT_CHUNKS
        for k in range(XT_CHUNKS):
            nc.gpsimd.dma_start(out=xt[:, ds(k * ck, ck)],
                                in_=io["xt"][:, ds(k * ck, ck)])
        # [I I I] tiled identity (C copies of eye(128))
        id_t = cpool.tile([P, C * P], BF16, tag="ident", name="ident")
        nc.gpsimd.dma_start(out=id_t, in_=io["ident"])
        wf_t = []
        for d_t in range(n_dt):
            t = cpool.tile([P, O], BF16, tag=f"wf{d_t}", name=f"wf{d_t}")
            nc.gpsimd.dma_start(out=t, in_=io["wf"][ts(d_t, P), :])
            wf_t.append(t)
        # Wa rows / bias broadcast across partitions for the rank-1 epilogue
        wa_bc = []
        for c in range(C):
            t = cpool.tile([P, O], BF16, tag=f"wab{c}", name=f"wab{c}")
            nc.gpsimd.dma_start(
                out=t, in_=io["wa"][c:c + 1, :].to_broadcast((P, O)))
            wa_bc.append(t)
        bias_bc = cpool.tile([P, O], BF16, tag="bias", name="bias")
        nc.gpsimd.dma_start(
            out=bias_bc, in_=io["bias"].unsqueeze(0).to_broadcast((P, O)))
        cst.update(wf_t=wf_t, id_t=id_t, xt=xt, wa_bc=wa_bc, bias_bc=bias_bc)

        # shared PSUM: 2 gt + 4 m + 2 f = 8 banks
        cst["gt_psum"] = [accpool.tile([P, Y], F32, tag=f"gtp{d}", name=f"gtp{d}")
                          for d in range(n_dt)]
        cst["m_psum"] = [accpool.tile([P, C * P], F32, tag=f"mp{t}", name=f"mp{t}")
                         for t in range(n_yt)]
        cst["f_psum"] = [accpool.tile([P, n_yt * O // 2], F32, tag=f"fp{b}",
                                      name=f"fp{b}") for b in range(2)]

        for _ in range(reps):
            emit_body(tc, io, cst, N, Y, D, O, C)


def build_nc(N=N_FULL, Y=Y_FULL, D=D_FULL, O=O_FULL, C=C_FULL, reps=1,
             internal_inputs=False):
    nc = bacc.Bacc("TRN2", target_bir_lowering=False, debug=False)
    kind = "Internal" if internal_inputs else "ExternalInput"
    _, sub = _layout(Y, D, C)
    io = {
        "pkd": nc.dram_tensor("pkd", [P, (N // P) * sub], U8, kind=kind).ap(),
        "xt": nc.dram_tensor("xt", [P, (N // P) * D], U8, kind=kind).ap(),
        "wf": nc.dram_tensor("wf", [D, O], BF16, kind=kind).ap(),
        "wa": nc.dram_tensor("wa", [C, O], BF16, kind=kind).ap(),
        "bias": nc.dram_tensor("bias", [O], BF16, kind=kind).ap(),
        "ident": nc.dram_tensor("ident", [P, C * P], BF16, kind=kind).ap(),
        "out": nc.dram_tensor("out", [Y, O], F32, kind="ExternalOutput").ap(),
    }
    with tile.TileContext(nc) as tc:
        emit_kernel(tc, io, N, Y, D, O, C, reps=reps)
    nc.compile()
    return nc


_NC_CACHE = None


def _get_nc():
    global _NC_CACHE
    if _NC_CACHE is None:
        _NC_CACHE = build_nc()
    return _NC_CACHE


E4M3 = ml_dtypes.float8_e4m3


def _q8(a):
    return np.ascontiguousarray(a).astype(E4M3)


def run(adjacency, input_feature, Beta, weight, bias, trace=False):
    nc = _get_nc()
    adjacency = np.asarray(adjacency, dtype=np.float32)
    input_feature = np.asarray(input_feature, dtype=np.float32)
    Beta = np.asarray(Beta, dtype=np.float32)
    weight = np.ascontiguousarray(np.asarray(weight, dtype=np.float32))
    bias = np.ascontiguousarray(np.asarray(bias, dtype=np.float32))

    x8 = _q8(input_feature)
    # xt[p, s*D + d] = X8[s*128 + p, d]
    xt = np.ascontiguousarray(
        x8.view(np.uint8).reshape(N_FULL // P, P, D_FULL)
          .transpose(1, 0, 2).reshape(P, -1))
    wf = np.ascontiguousarray(weight[:D_FULL]).astype(ml_dtypes.bfloat16)
    wa = np.ascontiguousarray(weight[D_FULL:]).astype(ml_dtypes.bfloat16)
    bias16 = bias.astype(ml_dtypes.bfloat16)
    ident = np.ascontiguousarray(
        np.tile(np.eye(P, dtype=ml_dtypes.bfloat16), (1, C_FULL)))

    in_maps = []
    for i in range(N_CORES):
        ys = slice(i * Y_FULL, (i + 1) * Y_FULL)
        beta8 = _q8(Beta[:, ys])
        # [n][t][c][128] so one matmul per y-block covers all 3 channels
        adj8 = _q8(adjacency[:, ys, :]
                   .reshape(N_FULL, Y_FULL // P, P, C_FULL)
                   .transpose(0, 1, 3, 2)).reshape(N_FULL, -1)
        row = np.concatenate(
            [beta8.view(np.uint8), adj8.view(np.uint8)], axis=1)  # [N, sub]
        sub = row.shape[1]
        # pkd[p, s*sub + b] = row[s*128 + p, b]: any tile-aligned group is
        # a contiguous column slice
        pkd = np.ascontiguousarray(
            row.reshape(N_FULL // P, P, sub)
               .transpose(1, 0, 2).reshape(P, -1))
        in_maps.append({
            "pkd": pkd,
            "xt": xt,
            "wf": wf,
            "wa": wa,
            "bias": bias16,
            "ident": ident,
        })
    res = run_bass_kernel_spmd(nc, in_maps, core_ids=list(range(N_CORES)),
                               trace=trace)
    out = np.concatenate([res.results[i]["out"] for i in range(N_CORES)],
                         axis=0).astype(np.float32)
    return out, res


def kernel(adjacency, input_feature, Beta, weight, bias):
    out, _ = run(adjacency, input_feature, Beta, weight, bias, trace=False)
    return out


# revision 53
# speedup vs baseline: 10.6450x; 2.3680x over previous
"""Trainium2 Bass kernel for nn_CausalAggregator.

Computes, for target stocks y:
    out[y, :] = Beta[:, y] @ concat([X, adjacency[:, y, :]], 1) @ W + bias
              = (Beta.T @ X) @ Wf  +  (einsum('ny,nyc->yc', Beta, adj)) @ Wa + bias

Sharding: split Beta / adjacency along the target axis y across 8 cores;
replicate X, weight, bias. Each core computes 512 output rows; no
cross-device reduction.

Per-core algorithm (N=4096 source stocks, Y=512 targets, D=O=256, C=3).
The kernel is HBM-bound (DMA queues share one ~360 GB/s pool), so every
wire tensor is downcast host-side (free) to fp8-e4m3.  X is loop-invariant
and replicated, so it is hoisted into SBUF once as a const; the per-rep
stream is row n = [beta(512B) | adj channel-major(1536B)], grouped GS n-tiles
per DMA with a tile-major permutation (contraction is order-invariant in n).
~8.4 MB/core vs 38 MB fp32.

All matmuls run in fp8 DoubleRow perf mode (K=256/pass, 0.5 cyc/row).
The einsum term runs on the PE as t-major diagonal-block matmuls: for each
y-block t, M_t += Beta_t^T @ [A0|A1|A2]_t accumulates a [128, 384] PSUM
tile whose three 128-col diagonals are the per-target channel sums; the
epilogue extracts them with one tiled-identity mask mul (DVE) per block,
then lands the adjacency term directly into the output accumulation as
rank-1 diag-mask @ broadcast-Wa matmuls (bias likewise via identity @
broadcast-bias), so there is no selector/stacking stage at all.

y-blocks 0,2 accumulate in one PSUM bank and 1,3 in another so a block's
drain never serializes the next block; one strided copy + one output DMA
per bank (Activation/DVE) writes the rep's result.  PSUM start=True
zeroes the whole 2KB bank, so only the FIRST matmul touching each bank
per rep carries start=True (M_START_ZERO) instead of DVE memsets.

Queue discipline: the big input-stream DMAs ride the sync/scalar (HWDGE)
queues, consts ride the Pool queue, and output DMAs reuse sync/scalar
(idle by the tail).  The DMA engines serialize at ~360 GB/s, so bytes ~=
time; IO_BUFS input buffering keeps them saturated across rep
boundaries.  The exec backend also charges roughly per instruction, so
the kernel favors few, large ops (GS=16 -> 2 stream DMAs/rep, ~130
instructions/rep total).  Constants and PSUM accumulators are hoisted
out of the per-rep body so back-to-back invocations pipeline.
"""

import numpy as np
import ml_dtypes

import concourse.mybir as mybir
import concourse.tile as tile
from concourse import bacc
from concourse.bass import ds, ts
from concourse.bass_utils import run_bass_kernel_spmd

P = 128
F32 = mybir.dt.float32
BF16 = mybir.dt.bfloat16
F8 = mybir.dt.float8e4
F16 = mybir.dt.float16
U8 = mybir.dt.uint8
DR = mybir.MatmulPerfMode.DoubleRow
COPY = mybir.ActivationFunctionType.Copy

# Full problem shapes (hardcoded; kernel.py must be self-contained).
N_FULL = 4096   # source stocks (contraction axis)
Y_TOTAL = 4096  # target stocks (sharded)
D_FULL = 256    # input features
O_FULL = 256    # output features
C_FULL = 3      # adjacency channels (weight rows = input_dim + 3)
N_CORES = 8
Y_FULL = Y_TOTAL // N_CORES  # per-core target slice
GS = 16         # n-tiles per DMA group (GSCHED overrides when set; mixed
# sizes let the tile scheduler reorder DMAs and wreck arrival order, so
# keep it uniform).  The exec backend charges ~0.5us per DMA instruction,
# so fewer/bigger stream DMAs win; GS=32 (one DMA/rep) loses pipelining.
GSCHED = None
IO_BUFS = 3     # input stream multi-buffering
XT_CHUNKS = 4   # const X loads split so the first group isn't gated on 1MB

M_START_ZERO = True  # zero m_psum via first-matmul start=True bank clear
# Keep-warm fillers help the cost MODEL (p-state hold) but the actual exec
# backend charges ~70ns per instruction, which swamps the modeled gain —
# measured 46us/rep with 360 fillers vs 20us/rep without.  Leave at 0.
N_FILL = 0
OUT_Q = "hw"    # output DMA queues: "hw" = sync/scalar HWDGE, "pool" = gpsimd
DBG_SKIP = frozenset()  # sim-only ablation: {"gt", "m", "epi", "f"}


def _layout(Y, D, C):
    """Byte offsets of the packed subtile row [beta | adj]."""
    off, lay = 0, {}
    for name, sz in (("beta", Y), ("adj", C * Y)):
        lay[name] = off
        off += sz
    return lay, off


def emit_body(tc, io, cst, N, Y, D, O, C):
    nc = tc.nc
    pkd, out = io["pkd"], io["out"]
    n_nt, n_yt, n_dt = N // P, Y // P, D // P
    lay, sub = _layout(Y, D, C)
    sched = GSCHED or [GS] * (n_nt // GS)
    assert sum(sched) == n_nt and all(g % 2 == 0 for g in sched)
    n_grp = len(sched)
    n_pairs = n_nt // 2

    iopool, fpool, opool = cst["iopool"], cst["fpool"], cst["opool"]
    gt_psum, m_psum, f_psum = cst["gt_psum"], cst["m_psum"], cst["f_psum"]
    wf_t, wa_bc, bias_bc, id_t, xt = (
        cst["wf_t"], cst["wa_bc"], cst["bias_bc"], cst["id_t"], cst["xt"])

    gt_sb = [fpool.tile([P, Y], BF16, tag=f"gt{d_t}", name=f"gt{d_t}")
             for d_t in range(n_dt)]
    mask_sb = [fpool.tile([P, C * P], BF16, tag=f"mask{t}", name=f"mask{t}")
               for t in range(n_yt)]

    if not M_START_ZERO:
        for t in range(n_yt):
            nc.vector.memset(m_psum[t], 0.0)

    queues = [nc.sync, nc.scalar]
    s0 = 0
    for g, gs in enumerate(sched):
        pk_t = iopool.tile([P, gs * sub], U8, tag=f"pk{gs}", name=f"pk{gs}")
        queues[g % 2].dma_start(out=pk_t, in_=pkd[:, ds(s0 * sub, gs * sub)])

        for jj in range(gs // 2):
            pi = s0 // 2 + jj
            first = pi == 0
            last = pi == n_pairs - 1
            pair = pk_t[:, ds(jj * 2 * sub, 2 * sub)].rearrange(
                "p (i b) -> p i b", i=2)
            beta_p = pair[:, :, ds(lay["beta"], Y)].bitcast(F8)
            adj_p = pair[:, :, ds(lay["adj"], C * Y)].bitcast(F8)
            x_p = xt[:, ds(pi * 2 * D, 2 * D)].rearrange(
                "p (i b) -> p i b", i=2).bitcast(F8)

            if "gt" not in DBG_SKIP:
                for d_t in range(n_dt):
                    nc.tensor.matmul(gt_psum[d_t], x_p[:, :, ts(d_t, P)],
                                     beta_p, start=first, stop=last,
                                     perf_mode=DR)
            if "m" not in DBG_SKIP:
                # t-major: one matmul per y-block covers all 3 channels
                # (adj is packed [t][c][128] within each row)
                for t in range(n_yt):
                    nc.tensor.matmul(
                        m_psum[t][:, 0:C * P],
                        beta_p[:, :, ts(t, P)],
                        adj_p[:, :, ds(t * C * P, C * P)],
                        start=(M_START_ZERO and first),
                        stop=last, perf_mode=DR,
                        skip_group_check=True)
            # keep-warm fillers: PE idles waiting on the next group's DMA,
            # which drops it out of max p-state; cheap no-dep matmuls into
            # the (dead until epilogue) f banks keep the clock up.  None
            # after the last group so the real tail is never delayed.
            if "f" not in DBG_SKIP and jj == gs // 2 - 1 and g < n_grp - 2:
                for k in range(N_FILL):
                    nc.tensor.matmul(
                        f_psum[g % 2][:, ds((k % 4) * P, P)],
                        x_p[:, :, 0:P], x_p[:, :, 0:P], start=True,
                        stop=False, perf_mode=DR, skip_group_check=True)
        s0 += gs

    if "epi" in DBG_SKIP:
        return
    # drain accumulators to SBUF: gt on Activation, per-y-block masks on
    # DVE so each output block unblocks as early as possible
    for d_t in range(n_dt):
        nc.scalar.activation(gt_sb[d_t], gt_psum[d_t], COPY)
    for t in range(n_yt):
        nc.vector.tensor_mul(mask_sb[t], m_psum[t][:, 0:C * P],
                             id_t[:, 0:C * P])

    # y-blocks 0,2 accumulate in psum bank A, 1,3 in bank B so a block's
    # drain never serializes against the next block's matmuls; the first
    # matmul touching each bank carries start=True (whole-bank zero).  The
    # adjacency term lands as rank-1 diag-mask @ broadcast-Wa matmuls; bias
    # likewise via identity @ broadcast-bias.
    o_sb = opool.tile([P, n_yt * O], F32, tag="osb", name="osb")
    # per y-half: bias+feat matmuls (gated on that half's gt drain), then
    # the mask-gated adjacency rank-1 terms, copy, and per-block output
    # DMA — so PE's in-order queue never parks later-gated work in front
    # of an unblocked block
    for half in range(2):
        for y_t in (2 * half, 2 * half + 1):
            fp = f_psum[y_t % 2][:, ds((y_t // 2) * O, O)]
            nc.tensor.matmul(fp, id_t[:, 0:P], bias_bc,
                             start=(y_t < 2), stop=False,
                             skip_group_check=True)
            for d_t in range(n_dt):
                nc.tensor.matmul(fp, gt_sb[d_t][:, ts(y_t, P)],
                                 wf_t[d_t], start=False,
                                 stop=False, skip_group_check=True)
        for y_t in (2 * half, 2 * half + 1):
            fp = f_psum[y_t % 2][:, ds((y_t // 2) * O, O)]
            for c in range(C):
                nc.tensor.matmul(fp, mask_sb[y_t][:, ts(c, P)], wa_bc[c],
                                 start=False, stop=(c == C - 1),
                                 skip_group_check=True)
    # one copy + one output DMA per psum bank (bank b holds y-blocks b and
    # b+2): the exec backend charges per instruction, so fewer, bigger ops
    for b in range(2):
        osl = o_sb.rearrange("p (t o) -> p t o", t=n_yt)[:, b::2, :]
        if b:
            nc.vector.tensor_copy(osl, f_psum[b].rearrange(
                "p (t o) -> p t o", t=2))
        else:
            nc.scalar.activation(osl, f_psum[b].rearrange(
                "p (t o) -> p t o", t=2), COPY)
        oq = ((nc.sync, nc.scalar) if OUT_Q == "hw"
              else (nc.gpsimd, nc.gpsimd))[b]
        oq.dma_start(
            out=out.rearrange("(t p) o -> p t o", p=P)[:, b::2, :], in_=osl)


def emit_kernel(tc, io, N, Y, D, O, C, reps=1):
    nc = tc.nc
    n_dt = D // P
    n_nt = N // P
    n_yt = Y // P
    with (
        tc.tile_pool(name="const", bufs=1) as cpool,
        tc.tile_pool(name="io", bufs=IO_BUFS) as iopool,
        tc.tile_pool(name="fin", bufs=2) as fpool,
        tc.tile_pool(name="osb", bufs=2) as opool,
        tc.tile_pool(name="acc", bufs=1, space="PSUM") as accpool,
    ):
        cst = {"iopool": iopool, "fpool": fpool, "opool": opool}
        # X^T stream hoisted into SBUF once: [128, n_nt * D] fp8 bytes,
        # subtile s holds rows [s*128, (s+1)*128) of X
        xt = cpool.tile([P, n_nt * D], U8, tag="xt", name="xt")
        ck = n_nt * D // XT_CHUNKS
        for k in range(XT_CHUNKS):
            nc.gpsimd.dma_start(out=xt[:, ds(k * ck, ck)],
                                in_=io["xt"][:, ds(k * ck, ck)])
        # [I I I] tiled identity (C copies of eye(128))
        id_t = cpool.tile([P, C * P], BF16, tag="ident", name="ident")
        nc.gpsimd.dma_start(out=id_t, in_=io["ident"])
        wf_t = []
        for d_t in range(n_dt):
            t = cpool.tile([P, O], BF16, tag=f"wf{d_t}", name=f"wf{d_t}")
            nc.gpsimd.dma_start(out=t, in_=io["wf"][ts(d_t, P), :])
            wf_t.append(t)
        # Wa rows / bias broadcast across partitions for the rank-1 epilogue
        wa_bc = []
        for c in range(C):
            t = cpool.tile([P, O], BF16, tag=f"wab{c}", name=f"wab{c}")
            nc.gpsimd.dma_start(
                out=t, in_=io["wa"][c:c + 1, :].to_broadcast((P, O)))
            wa_bc.append(t)
        bias_bc = cpool.tile([P, O], BF16, tag="bias", name="bias")
        nc.gpsimd.dma_start(
            out=bias_bc, in_=io["bias"].unsqueeze(0).to_broadcast((P, O)))
        cst.update(wf_t=wf_t, id_t=id_t, xt=xt, wa_bc=wa_bc, bias_bc=bias_bc)

        # shared PSUM: 2 gt + 4 m + 2 f = 8 banks
        cst["gt_psum"] = [accpool.tile([P, Y], F32, tag=f"gtp{d}", name=f"gtp{d}")
                          for d in range(n_dt)]
        cst["m_psum"] = [accpool.tile([P, C * P], F32, tag=f"mp{t}", name=f"mp{t}")
                         for t in range(n_yt)]
        cst["f_psum"] = [accpool.tile([P, n_yt * O // 2], F32, tag=f"fp{b}",
                                      name=f"fp{b}") for b in range(2)]

        for _ in range(reps):
            emit_body(tc, io, cst, N, Y, D, O, C)


def build_nc(N=N_FULL, Y=Y_FULL, D=D_FULL, O=O_FULL, C=C_FULL, reps=1,
             internal_inputs=False):
    nc = bacc.Bacc("TRN2", target_bir_lowering=False, debug=False)
    kind = "Internal" if internal_inputs else "ExternalInput"
    _, sub = _layout(Y, D, C)
    io = {
        "pkd": nc.dram_tensor("pkd", [P, (N // P) * sub], U8, kind=kind).ap(),
        "xt": nc.dram_tensor("xt", [P, (N // P) * D], U8, kind=kind).ap(),
        "wf": nc.dram_tensor("wf", [D, O], BF16, kind=kind).ap(),
        "wa": nc.dram_tensor("wa", [C, O], BF16, kind=kind).ap(),
        "bias": nc.dram_tensor("bias", [O], BF16, kind=kind).ap(),
        "ident": nc.dram_tensor("ident", [P, C * P], BF16, kind=kind).ap(),
        "out": nc.dram_tensor("out", [Y, O], F32, kind="ExternalOutput").ap(),
    }
    with tile.TileContext(nc) as tc:
        emit_kernel(tc, io, N, Y, D, O, C, reps=reps)
    nc.compile()
    return nc


_NC_CACHE = None


def _get_nc():
    global _NC_CACHE
    if _NC_CACHE is None:
        _NC_CACHE = build_nc()
    return _NC_CACHE


E4M3 = ml_dtypes.float8_e4m3


def _q8(a):
    return np.ascontiguousarray(a).astype(E4M3)


def run(adjacency, input_feature, Beta, weight, bias, trace=False):
    nc = _get_nc()
    adjacency = np.asarray(adjacency, dtype=np.float32)
    input_feature = np.asarray(input_feature, dtype=np.float32)
    Beta = np.asarray(Beta, dtype=np.float32)
    weight = np.ascontiguousarray(np.asarray(weight, dtype=np.float32))
    bias = np.ascontiguousarray(np.asarray(bias, dtype=np.float32))

    x8 = _q8(input_feature)
    # xt[p, s*D + d] = X8[s*128 + p, d]
    xt = np.ascontiguousarray(
        x8.view(np.uint8).reshape(N_FULL // P, P, D_FULL)
          .transpose(1, 0, 2).reshape(P, -1))
    wf = np.ascontiguousarray(weight[:D_FULL]).astype(ml_dtypes.bfloat16)
    wa = np.ascontiguousarray(weight[D_FULL:]).astype(ml_dtypes.bfloat16)
    bias16 = bias.astype(ml_dtypes.bfloat16)
    ident = np.ascontiguousarray(
        np.tile(np.eye(P, dtype=ml_dtypes.bfloat16), (1, C_FULL)))

    in_maps = []
    for i in range(N_CORES):
        ys = slice(i * Y_FULL, (i + 1) * Y_FULL)
        beta8 = _q8(Beta[:, ys])
        # [n][t][c][128] so one matmul per y-block covers all 3 channels
        adj8 = _q8(adjacency[:, ys, :]
                   .reshape(N_FULL, Y_FULL // P, P, C_FULL)
                   .transpose(0, 1, 3, 2)).reshape(N_FULL, -1)
        row = np.concatenate(
            [beta8.view(np.uint8), adj8.view(np.uint8)], axis=1)  # [N, sub]
        sub = row.shape[1]
        # pkd[p, s*sub + b] = row[s*128 + p, b]: any tile-aligned group is
        # a contiguous column slice
        pkd = np.ascontiguousarray(
            row.reshape(N_FULL // P, P, sub)
               .transpose(1, 0, 2).reshape(P, -1))
        in_maps.append({
            "pkd": pkd,
            "xt": xt,
            "wf": wf,
            "wa": wa,
            "bias": bias16,
            "ident": ident,
        })
    res = run_bass_kernel_spmd(nc, in_maps, core_ids=list(range(N_CORES)),
                               trace=trace)
    out = np.concatenate([res.results[i]["out"] for i in range(N_CORES)],
                         axis=0).astype(np.float32)
    return out, res


def kernel(adjacency, input_feature, Beta, weight, bias):
    out, _ = run(adjacency, input_feature, Beta, weight, bias, trace=False)
    return out
